# revision 6
# baseline (speedup 1.0000x reference)
"""Bass/Trainium2 kernel for nn_LSTMModel (B=128, T=512, D=256, H=512).

Sharding: data-parallel over batch across 8 NeuronCores (16 rows each),
weights replicated and SBUF-resident in bf16 transposed (lhsT) layout.

Scan: weight-stationary matmuls, gates on PSUM partitions.
Gate g = mc*128 + p; gate types i = mc 0..3, f = 4..7, g = 8..11, o = 12..15.
h/c state layout [128, (j, b)] with h-index = j*128 + p, so h slices
[:, j*16:(j+1)*16] are directly the K-chunk rhs of the next matmul.

v1 restructure (vs v0's xg-precompute pipeline): everything that used to
be DVE work rides the PE instead —
- per-gate bias enters each PSUM accumulation group as a K=1 rank-1
  matmul (flat [1, 4H] bias row x ones[1,16]), start=True.
- the x-side (xp @ wx0.T) is accumulated straight into the same PSUM
  group per step (no xg SBUF staging, no DVE add on the critical path).
- L0 gates split over two PSUM banks [g|i] then [f|o] so the sigmoid
  starts after half the h-side matmuls (same trick as L1).
- tanh(g) = 2*sigmoid(2g) - 1 with g-weights (and g-bias) pre-scaled 2x
  so one sigmoid ACT covers a whole bank.
- the final h = sig_o * tanh(c) mul runs on GPSIMD to keep DVE off the
  loop-carried path.
- L1 runs one step behind L0 (software pipeline) so each chain's latency
  hides under the other layer's matmul stream.
"""

import numpy as np

import concourse.bass as bass
import concourse.tile as tile
import concourse.mybir as mybir
from concourse import bacc
from concourse.bass import ds
from concourse.bass_utils import run_bass_kernel_spmd
from concourse.masks import make_identity

F32 = mybir.dt.float32
BF16 = mybir.dt.bfloat16
AF = mybir.ActivationFunctionType
OP = mybir.AluOpType

B, T, D, H = 128, 512, 256, 512
NCORES = 8
BL = B // NCORES            # 16
G = 4 * H                   # 2048
MCH = G // 128              # 16
DKC = D // 128              # 2
HKC = H // 128              # 4

# Bank A = [g|i] (tanh path starts early), bank B = [f|o].
A_MCS = [8, 9, 10, 11, 0, 1, 2, 3]
B_MCS = [4, 5, 6, 7, 12, 13, 14, 15]


def build_nc(t_steps=T, unroll=16, mode="real", staggered=True):
    assert t_steps % (2 * unroll) == 0
    ntot = t_steps * BL
    nch = 512 if ntot % 512 == 0 else ntot

    nc = bacc.Bacc("TRN2", target_bir_lowering=False)

    x = nc.dram_tensor("x", [BL, t_steps, D], F32, kind="ExternalInput")
    proj_w = nc.dram_tensor("proj_w", [D, D], F32, kind="ExternalInput")
    proj_b = nc.dram_tensor("proj_b", [D], F32, kind="ExternalInput")
    wx0 = nc.dram_tensor("wx0", [G, D], F32, kind="ExternalInput")
    bx0 = nc.dram_tensor("bx0", [G], F32, kind="ExternalInput")
    wh0 = nc.dram_tensor("wh0", [G, H], F32, kind="ExternalInput")
    bh0 = nc.dram_tensor("bh0", [G], F32, kind="ExternalInput")
    wx1 = nc.dram_tensor("wx1", [G, H], F32, kind="ExternalInput")
    bx1 = nc.dram_tensor("bx1", [G], F32, kind="ExternalInput")
    wh1 = nc.dram_tensor("wh1", [G, H], F32, kind="ExternalInput")
    bh1 = nc.dram_tensor("bh1", [G], F32, kind="ExternalInput")
    fc1_w = nc.dram_tensor("fc1_w", [32, H], F32, kind="ExternalInput")
    fc1_b = nc.dram_tensor("fc1_b", [32], F32, kind="ExternalInput")
    fc2_w = nc.dram_tensor("fc2_w", [1, 32], F32, kind="ExternalInput")
    fc2_b = nc.dram_tensor("fc2_b", [1], F32, kind="ExternalInput")
    out_d = nc.dram_tensor("out", [BL, 1], F32, kind="ExternalOutput")

    tens = dict(locals())
    with tile.TileContext(nc) as tc:
        with tc.tile_pool(name="res", bufs=1) as res, \
             tc.tile_pool(name="stg", bufs=3) as stg, \
             tc.tile_pool(name="scn", bufs=3) as scn, \
             tc.tile_pool(name="psum", bufs=2, space="PSUM") as psum:
            _build_body(nc, tc, res, stg, scn, psum, tens, t_steps,
                        unroll, ntot, nch, mode, staggered)
    nc.compile()
    return nc


def _build_body(nc, tc, res, stg, scn, psum, tens, t_steps, unroll, ntot,
                nch, mode, staggered):
    x, out_d = tens["x"], tens["out_d"]
    ublk = unroll * 16          # xp columns consumed per unrolled block

    ident = res.tile([128, 128], F32, tag="ident")
    make_identity(nc, ident[:, :])

    # ---- resident transposed weights (bf16); g-gate rows pre-scaled 2x ----
    w0T = res.tile([128, 6 * G], BF16, tag="w0T")    # kc 0..1 wx0, 2..5 wh0
    w1T = res.tile([128, 8 * G], BF16, tag="w1T")    # kc 0..3 wx1, 4..7 wh1
    for w_d, kcs, dst, kbase in ((tens["wx0"], DKC, w0T, 0),
                                 (tens["wh0"], HKC, w0T, DKC),
                                 (tens["wx1"], HKC, w1T, 0),
                                 (tens["wh1"], HKC, w1T, HKC)):
        cdim = w_d.shape[1]
        for gc in range(MCH):
            st = stg.tile([128, 512], F32, tag="wstage")
            nc.sync.dma_start(out=st[:, 0:cdim],
                              in_=w_d[gc * 128:(gc + 1) * 128, :])
            for kc in range(kcs):
                pt = psum.tile([128, 512], F32, tag="big")
                nc.tensor.transpose(pt[:, 0:128],
                                    st[:, kc * 128:(kc + 1) * 128],
                                    ident[:, :])
                o = ((kbase + kc) * MCH + gc) * 128
                if 8 <= gc <= 11:   # tanh(x) = 2*sigmoid(2x) - 1
                    nc.vector.tensor_scalar_mul(dst[:, o:o + 128],
                                                pt[:, 0:128], 2.0)
                else:
                    nc.vector.tensor_copy(dst[:, o:o + 128], pt[:, 0:128])

    projT = res.tile([128, 2 * D], F32, tag="projT")
    for gc in range(DKC):
        st = stg.tile([128, 512], F32, tag="wstage")
        nc.sync.dma_start(out=st[:, 0:D],
                          in_=tens["proj_w"][gc * 128:(gc + 1) * 128, :])
        for kc in range(DKC):
            pt = psum.tile([128, 512], F32, tag="big")
            nc.tensor.transpose(pt[:, 0:128],
                                st[:, kc * 128:(kc + 1) * 128], ident[:, :])
            nc.vector.tensor_copy(projT[:, (kc * 2 + gc) * 128:
                                        (kc * 2 + gc) * 128 + 128],
                                  pt[:, 0:128])

    fc1T = res.tile([128, HKC * 32], BF16, tag="fc1T")
    st = stg.tile([128, 512], F32, tag="wstage")
    nc.sync.dma_start(out=st[0:32, :], in_=tens["fc1_w"][:, :])
    for kc in range(HKC):
        pt = psum.tile([128, 512], F32, tag="big")
        nc.tensor.transpose(pt[:, 0:32], st[0:32, kc * 128:(kc + 1) * 128],
                            ident[0:32, 0:32])
        nc.vector.tensor_copy(fc1T[:, kc * 32:(kc + 1) * 32], pt[:, 0:32])
    fc2T_f = res.tile([32, 1], F32, tag="fc2T_f")
    nc.sync.dma_start(out=fc2T_f[:, :],
                      in_=tens["fc2_w"][0:1, :].rearrange("o k -> k o"))
    fc2T = res.tile([32, 1], BF16, tag="fc2T")
    nc.vector.tensor_copy(fc2T[:, :], fc2T_f[:, :])
    fc1b = res.tile([32, 1], F32, tag="fc1b")
    nc.sync.dma_start(out=fc1b[:, :],
                      in_=tens["fc1_b"][:].rearrange("(k o) -> k o", o=1))
    fc2b = res.tile([1, 1], F32, tag="fc2b")
    nc.sync.dma_start(out=fc2b[:, :],
                      in_=tens["fc2_b"][:].rearrange("(k o) -> k o", o=1))

    # ---- flat gate-bias rows [1, G] bf16 (g region pre-scaled 2x) ----
    # One partition holds the whole 4H bias vector so K=1 rank-1 matmuls
    # (bias row x ones) can inject bias into each PSUM group.
    brows = []
    for ba, bb in ((tens["bx0"], tens["bh0"]), (tens["bx1"], tens["bh1"])):
        sa = stg.tile([16, 128], F32, tag="bstage")
        nc.sync.dma_start(out=sa[:, :],
                          in_=ba[:].rearrange("(m p) -> m p", p=128))
        sb = stg.tile([16, 128], F32, tag="bstage")
        nc.sync.dma_start(out=sb[:, :],
                          in_=bb[:].rearrange("(m p) -> m p", p=128))
        tot = stg.tile([16, 128], BF16, tag="bsumst")
        nc.vector.tensor_add(tot[:, :], sa[:, :], sb[:, :])
        row = res.tile([1, G], BF16, tag=f"brow{len(brows)}")
        for m in range(MCH):
            nc.sync.dma_start(out=row[0:1, m * 128:(m + 1) * 128],
                              in_=tot[m:m + 1, :])
        # g gates (mc 8..11): scale 2x for the tanh-via-sigmoid trick
        # (2x is exact in bf16)
        nc.vector.tensor_scalar_mul(row[:, 8 * 128:12 * 128],
                                    row[:, 8 * 128:12 * 128], 2.0)
        brows.append(row)
    ones16 = res.tile([1, 16], BF16, tag="ones16")
    nc.vector.memset(ones16[:, :], 1.0)

    # ---- x -> xT (fp32, PE transpose), column order n = t*16 + b ----
    xT = res.tile([128, DKC * ntot], F32, tag="xT")
    for rc in range(t_steps // 8):
        stx = stg.tile([128, 256], F32, tag="xstage")
        for tt in range(8):
            nc.sync.dma_start(
                out=stx[tt * 16:(tt + 1) * 16, :].rearrange(
                    "p (o d) -> p o d", o=1),
                in_=x[:, rc * 8 + tt:rc * 8 + tt + 1, :])
        for kc in range(DKC):
            pt = psum.tile([128, 512], F32, tag="big")
            nc.tensor.transpose(pt[:, 0:128],
                                stx[:, kc * 128:(kc + 1) * 128], ident[:, :])
            nc.vector.tensor_copy(xT[:, kc * ntot + rc * 128:
                                     kc * ntot + rc * 128 + 128],
                                  pt[:, 0:128])

    stp = stg.tile([2, 128], F32, tag="bstage")
    nc.sync.dma_start(out=stp[0:2, :],
                      in_=tens["proj_b"][:].rearrange("(m p) -> m p", p=128))
    ptp = psum.tile([128, 512], F32, tag="big")
    nc.tensor.transpose(ptp[:, 0:2], stp[0:2, :], ident[0:2, 0:2])
    projb_t = res.tile([128, 2], F32, tag="projb")
    nc.vector.tensor_copy(projb_t[:, :], ptp[:, 0:2])

    # ---- xp = x @ proj_w.T + proj_b -> bf16 resident ----
    # padded by one block: the last speculative stage reads past T.
    ntot2 = ntot + ublk
    xp = res.tile([128, DKC * ntot2], BF16, tag="xp")
    for kc in range(DKC):
        nc.vector.memset(xp[:, kc * ntot2 + ntot:(kc + 1) * ntot2], 0.0)
    for nt in range(ntot // nch):
        for mc in range(DKC):
            px = psum.tile([128, 512], F32, tag="big")
            for kc in range(DKC):
                nc.tensor.matmul(
                    px[:, 0:nch],
                    projT[:, (kc * 2 + mc) * 128:(kc * 2 + mc) * 128 + 128],
                    xT[:, kc * ntot + nt * nch:kc * ntot + (nt + 1) * nch],
                    start=(kc == 0), stop=(kc == DKC - 1))
            nc.vector.tensor_scalar_add(
                xp[:, mc * ntot2 + nt * nch:mc * ntot2 + (nt + 1) * nch],
                px[:, 0:nch], projb_t[:, mc:mc + 1])

    # ---- scan state ----
    h0_dummy = res.tile([128, 64], BF16, tag="h0d")
    h1_dummy = res.tile([128, 64], BF16, tag="h1d")
    c0 = res.tile([128, 64], F32, tag="c0")
    c1 = res.tile([128, 64], F32, tag="c1")
    h0 = res.tile([128, 64], BF16, tag="h0")
    h1 = res.tile([128, 64], BF16, tag="h1")
    for s_ in (c0, c1, h0, h1, h0_dummy, h1_dummy):
        nc.vector.memset(s_[:, :], 0.0)

    # xp block staging ping-pong (bf16, both kc chunks of one block)
    xpA = res.tile([128, DKC * ublk], BF16, tag="xpA")
    xpB = res.tile([128, DKC * ublk], BF16, tag="xpB")
    xpv = xp[:, :].rearrange("p (k n) -> p k n", k=DKC)

    def stage_xp(dst, off):
        nc.sync.dma_start(
            out=dst[:, :].rearrange("p (k n) -> p k n", k=DKC),
            in_=xpv[:, :, ds(off, ublk)])

    def emit_l0(ps0, xpb, u):
        psA, psB = ps0[:, 0:128], ps0[:, 128:256]
        # One accumulation group for the whole [128,256] tile: the first
        # MM (start=True) marks the 2KB zero region pending-zero, every
        # later MM either initializes its cols or accumulates; the last
        # h-side MM of bank B stops the group. Bank A's cols are complete
        # after its own h-side MMs, so its sigmoid starts early (the race
        # tracker is range-based). Bias+x MMs are h0-independent and
        # emitted first to fill the PE while the previous chain finishes.
        for bi, (tile_, mcs) in enumerate(((psA, A_MCS), (psB, B_MCS))):
            for s, mc in enumerate(mcs):
                reg = tile_[:, s * 16:(s + 1) * 16]
                nc.tensor.matmul(
                    reg, brows[0][:, mc * 128:(mc + 1) * 128],
                    ones16[:, :], start=(bi == 0 and s == 0), stop=False)
                for kc in range(DKC):
                    nc.tensor.matmul(
                        reg,
                        w0T[:, (kc * MCH + mc) * 128:
                            (kc * MCH + mc) * 128 + 128],
                        xpb[:, kc * ublk + u * 16:kc * ublk + u * 16 + 16],
                        start=False, stop=False)
        for bi, (tile_, mcs) in enumerate(((psA, A_MCS), (psB, B_MCS))):
            for s, mc in enumerate(mcs):
                reg = tile_[:, s * 16:(s + 1) * 16]
                for j in range(HKC):
                    kc = DKC + j
                    nc.tensor.matmul(
                        reg,
                        w0T[:, (kc * MCH + mc) * 128:
                            (kc * MCH + mc) * 128 + 128],
                        h0[:, j * 16:(j + 1) * 16],
                        start=False,
                        stop=(bi == 1 and s == len(mcs) - 1
                              and j == HKC - 1))

    def emit_l1_bias(ps1):
        psA, psB = ps1[:, 0:128], ps1[:, 128:256]
        for bi, (tile_, mcs) in enumerate(((psA, A_MCS), (psB, B_MCS))):
            for s, mc in enumerate(mcs):
                nc.tensor.matmul(
                    tile_[:, s * 16:(s + 1) * 16],
                    brows[1][:, mc * 128:(mc + 1) * 128],
                    ones16[:, :], start=(bi == 0 and s == 0), stop=False)

    def emit_l1_side(ps1, src_h, kc_base, stop):
        psA, psB = ps1[:, 0:128], ps1[:, 128:256]
        for bi, (tile_, mcs) in enumerate(((psA, A_MCS), (psB, B_MCS))):
            for s, mc in enumerate(mcs):
                reg = tile_[:, s * 16:(s + 1) * 16]
                for j in range(HKC):
                    kc = kc_base + j
                    nc.tensor.matmul(
                        reg,
                        w1T[:, (kc * MCH + mc) * 128:
                            (kc * MCH + mc) * 128 + 128],
                        src_h[:, j * 16:(j + 1) * 16],
                        start=False,
                        stop=(stop and bi == 1 and s == len(mcs) - 1
                              and j == HKC - 1))

    def act(fn, dst, src_):
        nc.scalar.activation(dst, src_, fn)

    def chain(ps, cstate, hdst):
        psA, psB = ps[:, 0:128], ps[:, 128:256]
        # bank A = [g|i], bank B = [f|o]
        sga = scn.tile([128, 128], F32, tag="sga")
        sgb = scn.tile([128, 128], F32, tag="sgb")
        tg = scn.tile([128, 64], F32, tag="tg")
        tc_ = scn.tile([128, 64], F32, tag="tc")
        tmp = scn.tile([128, 64], F32, tag="tmp")
        act(AF.Sigmoid, sga[:, :], psA[:, :])
        nc.vector.tensor_scalar(tg[:, :], sga[:, 0:64], 2.0, 1.0,
                                OP.mult, OP.subtract)
        nc.vector.tensor_mul(tmp[:, :], sga[:, 64:128], tg[:, :])
        act(AF.Sigmoid, sgb[:, :], psB[:, :])
        nc.vector.tensor_mul(cstate[:, :], sgb[:, 0:64], cstate[:, :])
        nc.vector.tensor_add(cstate[:, :], cstate[:, :], tmp[:, :])
        act(AF.Tanh, tc_[:, :], cstate[:, :])
        nc.gpsimd.tensor_mul(hdst[:, :], sgb[:, 64:128], tc_[:, :])

    def l0_tile():
        return psum.tile([128, 256], F32, tag="ps0", name="ps0")

    def l1_tile():
        return psum.tile([128, 256], F32, tag="ps1", name="ps1")

    h0dst = h0 if mode == "real" else h0_dummy
    h1dst = h1 if mode == "real" else h1_dummy

    # ---- scan loop: L1 runs one step behind L0. Each subiter consumes a
    # staged xp block and prefetches the next into the other buffer.
    assert unroll == MCH
    n_iter = t_steps // unroll
    assert n_iter % 2 == 0
    stage_xp(xpA, 0)

    def subiter(cur_xp, nxt_xp, nxt_off):
        stage_xp(nxt_xp, nxt_off)
        prev_ps1 = None
        for u in range(unroll):
            ps0 = l0_tile()
            emit_l0(ps0, cur_xp, u)
            if prev_ps1 is not None:
                # step u-1's L1: h0-side reads h0(u-1), h1-side h1(u-2);
                # both must be emitted before chain(u) rewrites h0.
                emit_l1_bias(prev_ps1)
                emit_l1_side(prev_ps1, h0, 0, False)
                emit_l1_side(prev_ps1, h1, HKC, True)
            if mode != "nochain":
                chain(ps0, c0, h0dst)
            if prev_ps1 is not None and mode != "nochain":
                chain(prev_ps1, c1, h1dst)
            prev_ps1 = l1_tile()
        # epilogue: L1 of the last step of this subiteration
        emit_l1_bias(prev_ps1)
        emit_l1_side(prev_ps1, h0, 0, False)
        emit_l1_side(prev_ps1, h1, HKC, True)
        if mode != "nochain":
            chain(prev_ps1, c1, h1dst)

    with tc.For_i(0, n_iter // 2, 1,
                  hint_engines=(mybir.EngineType.PE,),
                  staggered_reset=staggered) as it:
        subiter(xpA, xpB, it * (2 * ublk) + ublk)
        subiter(xpB, xpA, it * (2 * ublk) + 2 * ublk)

    # ---- FC head ----
    ph = psum.tile([128, 512], F32, tag="big")
    for kc in range(HKC):
        nc.tensor.matmul(ph[0:32, 0:16], fc1T[:, kc * 32:(kc + 1) * 32],
                         h1[:, kc * 16:(kc + 1) * 16],
                         start=(kc == 0), stop=(kc == HKC - 1))
    hid = scn.tile([32, 16], BF16, tag="hid")
    nc.scalar.activation(hid[:, :], ph[0:32, 0:16], AF.Relu,
                         bias=fc1b[:, 0:1])
    po = psum.tile([128, 512], F32, tag="big")
    nc.tensor.matmul(po[0:1, 0:16], fc2T[:, 0:1], hid[:, :],
                     start=True, stop=True)
    ob = scn.tile([1, 16], F32, tag="ob")
    nc.vector.tensor_scalar_add(ob[:, :], po[0:1, 0:16], fc2b[0:1, 0:1])
    nc.sync.dma_start(out=out_d[:, :].rearrange("b o -> o b"), in_=ob[:, :])


_NC_CACHE = {}


def _get_nc(t_steps=T, unroll=16):
    key = (t_steps, unroll)
    if key not in _NC_CACHE:
        _NC_CACHE[key] = build_nc(t_steps, unroll, "real", staggered=True)
    return _NC_CACHE[key]


def kernel(**inputs):
    nc = _get_nc()
    arrs = {k: np.ascontiguousarray(np.asarray(v, dtype=np.float32))
            for k, v in inputs.items()}
    in_maps = []
    for c in range(NCORES):
        m = {k: v for k, v in arrs.items() if k != "x"}
        m["x"] = np.ascontiguousarray(arrs["x"][c * BL:(c + 1) * BL])
        in_maps.append(m)
    res = run_bass_kernel_spmd(nc, in_maps, core_ids=list(range(NCORES)))
    return np.concatenate([r["out"] for r in res.results], axis=0)
